# revision 47
# baseline (speedup 1.0000x reference)
"""3-layer GCN (BaseNet) on 8 Trainium2 NeuronCores.

Strategy (self-contained; hardcoded for N=100000, E=1.6M, D=128, LAT=32x3, G=1000):
 - Nodes sharded contiguously across 8 cores (12500/core); each core owns all
   edges whose dst lies in its shard.
 - Per core, shard nodes are sorted by in-degree so the edge aggregation
   becomes "rounds": round r gathers hs[src_r(n)] for the prefix of nodes with
   indeg > r, landing slot-aligned with the accumulator -> plain DVE adds,
   no scatter needed.
 - Gathers are 128-row indirect DMAs from a per-core HBM replica of the
   scaled feature table hs = deg^-1/2 * (h @ W); replicas built by AllGather.
 - Math identity used: agg + self = dis * (sum_e hs[src] + hs[own]); the
   edge coefficient dis[src]*dis[dst] factors into per-node pre/post scales.
 - Backward slicing: layer-2 output is only needed at readout nodes and their
   in-neighbors (~17K nodes); layer-3 only at the 1000 readout nodes. Cuts
   gather volume ~2.7x.
 - Readout (first node of each graph) + 2-layer MLP + log_softmax on device.

Perf (this session):
 - Indirect-gather descriptor generation on the Pool/GPSIMD engine is the
   bottleneck (~1us fixed cost per 128-row indirect DMA; ~1800 of them).
   Spreading the gathers round-robin over 2 SWDGE queues (NQSW=2,
   qPoolDynamic/qPoolDynamic1) parallelizes desc-gen on HW (~3.8x per-gather
   in microbench; NQ=4 measured worse).
 - Gather tables/stripes/AllGathers are bf16 (halves collective + gather
   bytes; sim: collectives 570->392us, DMA 200->133us). All accumulation
   stays fp32 (separate fp32 accumulator seeded from bf16 hs for layer 1);
   rel err vs fp64 reference: 2.4e-4 (budget 2e-2).
 - Layer-1 gathers (1472 of ~1800 indirect DMAs) now use the dma_gather
   GPSIMD ucode instead (needs gpsimd.load_library(mlp) +
   mybir.codegen_inst_isa_subclasses(nc); single_packet=False above 1024
   indices; one pre-allocated Pool reg per num_idxs value). int16 index
   range is solved by the QUAD trick: idx = table_row>>2 (<=25120) into the
   bf16 table viewed as 256B quad-cells [T/4, 128]; precomputed {0,1}
   quadrant masks (row&3, bc_mid-broadcast) select the valid 32 features via
   mult+add on DVE. Holes point at a zeroed quad (all P table pad rows must
   be zeroed or NaN*0 poisons the mask-mult). ~8K rows per instruction,
   ~31 instructions total for L1.
 - Measured (interleaved pipelined differential, quiet-phase min): 365-436us
   vs 1363us for the indirect-DMA version, vs 1952us session baseline.
   NQSW multi-queue turned out to be a no-op on the real kernel (walrus
   assigns queues by instruction type); left at 2, harmless.
"""
import os
import sys

for _p in ("/opt/trn_rl_repo", "/root/.axon_site/_ro/trn_rl_repo"):
    if os.path.isdir(_p) and _p not in sys.path:
        sys.path.insert(0, _p)

import ml_dtypes
import numpy as np

import concourse.bass as bass
import concourse.mybir as mybir
import concourse.tile as tile_mod
from concourse import library_config
from concourse.masks import make_identity
from concourse.vector_clock import ScopedClock

NC = 8
F = 32          # latent feature dim
P = 128
NQSW = 2        # SWDGE queues for indirect gathers (parallel desc-gen)

FP32 = mybir.dt.float32
BF16 = mybir.dt.bfloat16
I32 = mybir.dt.int32
I16 = mybir.dt.int16
AF = mybir.ActivationFunctionType
ALU = mybir.AluOpType

# ---------------------------------------------------------------------------
# walrus compat: this build rejects >1 sem wait per instruction. Spread the
# Tile drain's waits and any multi-wait instruction across nop carriers.
# ---------------------------------------------------------------------------

def _patched_drain_and_barrier(self, tick_clock, wait_clock):
    probe = self.nc.sync.nop(nofuse=True, hint="drain_wait_carrier")
    wait_clock.add_sem_waits(probe.ins, ScopedClock({None: tick_clock.global_clock}))
    si = probe.ins.sync_info
    waits = list(si.on_wait) if si is not None else []
    probe.ins.sync_info = mybir.SyncInfo(
        on_wait=waits[:1], on_update=list(si.on_update) if si is not None else []
    )
    for k in range(1, len(waits)):
        extra = self.nc.sync.nop(nofuse=True, hint=f"drain_wait_carrier_{k}")
        extra.ins.sync_info = mybir.SyncInfo(on_wait=waits[k : k + 1], on_update=[])
    self.nc.sync.drain()
    self.nc.all_engine_barrier()
    assert self.sems is not None
    popped = self.nc._tile_sem_poison_stack.pop()
    assert popped is self._sem_poison
    self.nc.clear_and_free_semaphores(list(self.sems.allocated().values()))
    self.nc.all_engine_barrier()


tile_mod.TileContext._drain_and_barrier = _patched_drain_and_barrier


def _split_waits(nc, max_waits=1):
    ctr = 0
    for fn in nc.m.functions:
        for bb in fn.blocks:
            out = []
            changed = False
            for inst in bb.instructions:
                si = inst.sync_info
                n = len(si.on_wait) if si is not None else 0
                if n > max_waits:
                    waits = list(si.on_wait)
                    keep = waits[-max_waits:]
                    extra = waits[:-max_waits]
                    for k in range(0, len(extra), max_waits):
                        nop = mybir.InstNoOp(
                            name=f"waitnop-{ctr}",
                            engine=inst.engine,
                            bass_nofuse=True,
                            sync_info=mybir.SyncInfo(
                                on_wait=extra[k : k + max_waits], on_update=[]
                            ),
                        )
                        ctr += 1
                        out.append(nop)
                    inst.sync_info = mybir.SyncInfo(
                        on_wait=keep, on_update=list(si.on_update)
                    )
                    changed = True
                out.append(inst)
            if changed:
                bb.instructions = out
    return ctr


# ---------------------------------------------------------------------------
# SPMD runner (PJRT/axon), build once / run many.
# ---------------------------------------------------------------------------

class _SpmdRunner:
    def __init__(self, nc, n_cores=NC):
        import jax
        from jax.sharding import Mesh, NamedSharding, PartitionSpec
        from jax.experimental.shard_map import shard_map
        from concourse.bass2jax import (
            _bass_exec_p,
            install_neuronx_cc_hook,
            partition_id_tensor,
        )

        self.jax = jax
        install_neuronx_cc_hook()
        self.n_cores = n_cores
        partition_name = nc.partition_id_tensor.name if nc.partition_id_tensor else None

        in_names, out_names, out_avals = [], [], []
        for alloc in nc.m.functions[0].allocations:
            if not isinstance(alloc, mybir.MemoryLocationSet):
                continue
            name = alloc.memorylocations[0].name
            if alloc.kind == "ExternalInput":
                if name != partition_name:
                    in_names.append(name)
            elif alloc.kind == "ExternalOutput":
                out_avals.append(
                    jax.core.ShapedArray(
                        tuple(alloc.tensor_shape), mybir.dt.np(alloc.dtype)
                    )
                )
                out_names.append(name)
        self.in_names, self.out_names, self.out_avals = in_names, out_names, out_avals
        n_params = len(in_names)
        all_in = in_names + out_names + ([partition_name] if partition_name else [])

        def _body(*args):
            operands = list(args)
            if partition_name is not None:
                operands.append(partition_id_tensor())
            return tuple(
                _bass_exec_p.bind(
                    *operands,
                    out_avals=tuple(out_avals),
                    in_names=tuple(all_in),
                    out_names=tuple(out_names),
                    lowering_input_output_aliases=(),
                    sim_require_finite=True,
                    sim_require_nnan=True,
                    nc=nc,
                )
            )

        devices = jax.devices()[:n_cores]
        assert len(devices) == n_cores, f"need {n_cores} cores, saw {len(jax.devices())}"
        mesh = Mesh(np.asarray(devices), ("core",))
        self.sharded = jax.jit(
            shard_map(
                _body,
                mesh=mesh,
                in_specs=(PartitionSpec("core"),) * (n_params + len(out_names)),
                out_specs=(PartitionSpec("core"),) * len(out_names),
                check_rep=False,
            ),
            keep_unused=True,
        )
        self.sharding = NamedSharding(mesh, PartitionSpec("core"))

    def stage(self, in_maps):
        args = []
        for name in self.in_names:
            cat = np.concatenate(
                [np.ascontiguousarray(in_maps[c][name]) for c in range(self.n_cores)],
                axis=0,
            )
            args.append(self.jax.device_put(cat, self.sharding))
        for av in self.out_avals:
            z = np.zeros((self.n_cores * av.shape[0], *av.shape[1:]), av.dtype)
            args.append(self.jax.device_put(z, self.sharding))
        return args

    def run_staged(self, args):
        outs = self.sharded(*args)
        self.jax.block_until_ready(outs)
        return outs

    def results(self, outs):
        n = self.n_cores
        return [
            {
                name: np.asarray(outs[i]).reshape(n, *self.out_avals[i].shape)[c]
                for i, name in enumerate(self.out_names)
            }
            for c in range(n)
        ]


# ---------------------------------------------------------------------------
# Host preprocessing
# ---------------------------------------------------------------------------

def _cdiv(a, b):
    return -(-a // b)


class _AggStruct:
    """Slot-aligned gather rounds for one layer, unified across cores."""

    def __init__(self, active, indeg, shard):
        N = active.shape[0]
        self.N = N
        self.shard = shard
        self.pos = np.full(N, -1, np.int64)
        self.perm = []
        counts = []
        for c in range(NC):
            nodes = np.flatnonzero(active[c * shard : (c + 1) * shard]) + c * shard
            order = np.argsort(-indeg[nodes], kind="stable")
            pc = nodes[order]
            self.perm.append(pc)
            self.pos[pc] = np.arange(len(pc))
            counts.append(len(pc))
        self.counts = counts
        self.chunks = max(1, _cdiv(max(counts), P))
        SPa = self.chunks * P
        self.SPa = SPa

        # rounds: cols per round, unified (max over cores)
        R = 0
        percore_d = []
        for c in range(NC):
            d = indeg[self.perm[c]]  # sorted desc
            R = max(R, int(d[0]) if len(d) else 0)
            percore_d.append(d)
        cols = []
        for r in range(R):
            m = 0
            for c in range(NC):
                nr = int(np.count_nonzero(percore_d[c] > r))
                m = max(m, _cdiv(nr, P))
            if m == 0:
                break
            cols.append(m)
        self.cols = cols
        self.NI = sum(cols)

    def build_idx(self, e_src, e_dst, t_row, Z):
        off = np.zeros(len(self.cols) + 1, np.int64)
        off[1:] = np.cumsum(self.cols)
        idx = np.full((NC, self.NI, P), Z, np.int32)
        if len(e_dst) and self.NI:
            key = (e_dst // self.shard) * self.SPa + self.pos[e_dst]
            order_e = np.argsort(key, kind="stable")
            sk = key[order_e]
            gs = np.r_[0, np.flatnonzero(np.diff(sk)) + 1]
            seq = np.arange(len(sk)) - np.repeat(gs, np.diff(np.r_[gs, len(sk)]))
            c_e = sk // self.SPa
            k_e = sk % self.SPa
            instr = off[seq] + (k_e // P)
            idx[c_e, instr, k_e % P] = t_row[e_src[order_e]].astype(np.int32)
        return idx  # [NC, NI, 128]

    def self_idx(self, t_row, Z):
        """[NC, chunks, 128] gather-own-row indices (for accumulator init)."""
        out = np.full((NC, self.chunks, P), Z, np.int32)
        for c in range(NC):
            k = np.arange(len(self.perm[c]))
            out[c, k // P, k % P] = t_row[self.perm[c]].astype(np.int32)
        return out

    def grid(self, vals, pad, dtype=np.float32):
        """[NC, 128, chunks] per-slot values in (p, ch) layout."""
        g = np.full((NC, P, self.chunks), pad, dtype)
        for c in range(NC):
            k = np.arange(len(self.perm[c]))
            g[c, k % P, k // P] = vals[self.perm[c]]
        return g

    def local_row(self, nodes):
        """flat stripe row (p*chunks + ch) of nodes within their own core."""
        pos = self.pos[nodes]
        return (pos % P) * self.chunks + pos // P

    def table_row(self, nodes):
        """global table row across core stripes."""
        c = np.asarray(nodes) // self.shard
        pos = self.pos[nodes]
        return c * self.SPa + (pos % P) * self.chunks + pos // P


def _preprocess(x, edge_index, batch, num_graphs):
    N = x.shape[0]
    E = edge_index.shape[1]
    G = int(num_graphs)
    shard = N // NC
    assert N % NC == 0
    src = edge_index[0].astype(np.int64)
    dst = edge_index[1].astype(np.int64)
    indeg = np.bincount(dst, minlength=N)
    deg = (indeg + 1).astype(np.float32)

    idxg = np.searchsorted(batch, np.arange(G, dtype=np.int64))

    # backward slicing
    act3 = np.zeros(N, bool)
    act3[idxg] = True
    m3 = act3[dst]
    act2 = np.zeros(N, bool)
    act2[src[m3]] = True
    act2[idxg] = True
    m2 = act2[dst]
    act1 = np.ones(N, bool)

    indeg2 = np.bincount(dst[m2], minlength=N)
    indeg3 = np.bincount(dst[m3], minlength=N)

    # layer 1: table covers all nodes, but only act1 nodes (h1 consumers:
    # sources of layer-2 edges + readout) aggregate. Sorting each shard as
    # [act1 by indeg desc, then inactive] keeps the accumulator == hs1 buffer
    # (self term free) while rounds cover only the active prefix.
    act1 = np.zeros(N, bool)
    act1[src[m2]] = True
    act1[idxg] = True
    m1 = act1[dst]
    indeg1m = np.where(act1, indeg, 0)
    a1 = _AggStruct(np.ones(N, bool), indeg1m, shard)
    Z1 = NC * a1.SPa
    t1 = a1.table_row(np.arange(N))
    idx1_raw = a1.build_idx(src[m1], dst[m1], t1, Z1)

    # layer 2: accumulate only at act2 nodes; table2 is full (t1 coords)
    a2 = _AggStruct(act2, indeg2, shard)
    idx2_raw = a2.build_idx(src[m2], dst[m2], t1, Z1)
    # layer 3: accumulate at readout nodes; table3 covers act2 (perm2 stripes)
    t3 = np.zeros(N, np.int64)
    a2n = np.flatnonzero(act2)
    t3[a2n] = a2.table_row(a2n)
    Z3 = NC * a2.SPa
    a3 = _AggStruct(act3, indeg3, shard)
    idx3_raw = a3.build_idx(src[m3], dst[m3], t3, Z3)

    self2 = a2.self_idx(t1, Z1)          # [NC, chunks2, 128]
    self3 = a3.self_idx(t3, Z3)

    # grids
    deg1_g = a1.grid(deg, 1.0)
    deg2_g = a2.grid(deg, 1.0)
    deg3_g = a3.grid(deg, 1.0)

    # x transposed+permuted per core: [128, SPa1]
    xT = np.zeros((NC, P, a1.SPa), np.float32)
    for c in range(NC):
        xT[c, :, : len(a1.perm[c])] = x[a1.perm[c]].T

    # readout: slot p of core c = a3.perm[c][p]
    r1_idx = np.zeros((NC, P, 1), np.int32)
    r2_idx = np.zeros((NC, P, 1), np.int32)
    graph_of = -np.ones(N, np.int64)
    graph_of[idxg] = np.arange(G)
    slot_graphs = []
    for c in range(NC):
        nodes = a3.perm[c]
        assert len(nodes) <= P, f"core {c} has {len(nodes)} graphs (>128)"
        r1_idx[c, : len(nodes), 0] = a1.local_row(nodes)
        r2_idx[c, : len(nodes), 0] = a2.local_row(nodes)
        slot_graphs.append(graph_of[nodes])

    # gather idx arrays, transposed to [128, NI] for contiguous DMA
    def tr(a):  # [NC, NI, 128] -> [NC, 128, NI]
        return np.ascontiguousarray(np.transpose(a, (0, 2, 1)))

    idx1 = tr(idx1_raw)
    idx2 = tr(np.concatenate([self2, idx2_raw], axis=1))
    idx3 = tr(np.concatenate([self3, idx3_raw], axis=1))
    if idx1.shape[2] == 0:  # degenerate: no edges at all
        idx1 = np.full((NC, P, 1), Z1, np.int32)
        a1.cols = [1]
        idx1_raw = np.full((NC, 1, P), Z1, np.int32)

    # quad-gather packing (dma_gather ucode): idx = row>>2 fits int16 for the
    # table viewed as 256B quad-cells; per-slot quadrant masks (row&3) select
    # the 32 valid features of the gathered 128. Every gather is padded to a
    # canonical chunk size in {4,8,16,32,64} cols with hole (Z) columns whose
    # output is never added -> only 5 distinct num_idxs registers needed.
    def quad_pack(raw, cols_list, Z):
        """raw [NC, NI, P] int32 table rows (holes = Z). Returns wrapped int16
        quad indices, quadrant masks, and per-round (padded, real) chunks."""
        def padto(c):
            for s in (4, 8, 16, 32, 64):
                if c <= s:
                    return s
            return 64

        chunks = []
        pieces = []
        off = 0
        for cols in cols_list:
            done = 0
            ch = []
            while done < cols:
                cc = min(64, cols - done)
                ccp = padto(cc)
                piece = np.full((NC, ccp, P), Z, np.int64)
                piece[:, :cc, :] = raw[:, off + done : off + done + cc, :]
                pieces.append(piece)
                ch.append((ccp, cc))
                done += cc
            chunks.append(tuple(ch))
            off += cols
        allr = np.concatenate(pieces, axis=1)  # [NC, NIp, P]
        NIp = allr.shape[1]
        flat = allr.reshape(NC, NIp * P)  # i = col*128 + p
        q = (flat >> 2).astype(np.int16)
        idxq = np.zeros((NC, P, NIp * 8), np.int16)
        for c in range(NC):
            blk = q[c].reshape(-1, 16).T
            for gq_ in range(8):
                idxq[c, gq_ * 16 : gq_ * 16 + 16, :] = blk
        par = allr & 3
        mq = np.zeros((NC, P, 4 * NIp), np.float32)
        for k in range(4):
            mq[:, :, k * NIp : (k + 1) * NIp] = np.transpose(par == k, (0, 2, 1))
        return idxq, mq.astype(ml_dtypes.bfloat16), tuple(chunks), NIp

    raw2 = np.concatenate([self2, idx2_raw], axis=1)
    raw3 = np.concatenate([self3, idx3_raw], axis=1)
    idxq1, mq1, chunks1, NI1q = quad_pack(idx1_raw, a1.cols, Z1)
    idxq2, mq2, chunks2, NI2q = quad_pack(raw2, [a2.chunks] + a2.cols, Z1)
    idxq3, mq3, chunks3, NI3q = quad_pack(raw3, [a3.chunks] + a3.cols, Z3)

    meta = dict(
        CH1=a1.chunks, CH2=a2.chunks, CH3=a3.chunks,
        SP1=a1.SPa, SP2=a2.SPa, SP3=a3.SPa,
        cols1=a1.cols,
        cols2=[a2.chunks] + a2.cols,
        cols3=[a3.chunks] + a3.cols,
        NI1=idx1.shape[2], NI2=idx2.shape[2], NI3=idx3.shape[2],
        G=G,
    )
    meta["NI1q"] = NI1q
    meta["NI2q"] = NI2q
    meta["NI3q"] = NI3q
    meta["chunks1"] = chunks1
    meta["chunks2"] = chunks2
    meta["chunks3"] = chunks3
    per_core = [
        dict(
            xT=xT[c], deg1=deg1_g[c], deg2=deg2_g[c], deg3=deg3_g[c],
            idxq1=idxq1[c], mq1=mq1[c], idx2=idx2[c], idx3=idx3[c],
            r1=r1_idx[c], r2=r2_idx[c],
        )
        for c in range(NC)
    ]
    return meta, per_core, slot_graphs


# ---------------------------------------------------------------------------
# Device program
# ---------------------------------------------------------------------------

def _build(meta, skip_gathers=False, skip_collectives=False):
    CH1, CH2, CH3 = meta["CH1"], meta["CH2"], meta["CH3"]
    SP1, SP2 = meta["SP1"], meta["SP2"]
    NI1, NI2, NI3 = meta["NI1"], meta["NI2"], meta["NI3"]
    T1_ROWS = NC * SP1 + P
    T3_ROWS = NC * SP2 + P

    nc = bass.Bass(num_swdge_queues=NQSW)
    qrr = [0]

    def _q(bi):
        """Round-robin indirect DMAs across SWDGE queues (parallel desc-gen)."""
        qi = qrr[0] % NQSW
        qrr[0] += 1
        if qi:
            bi.ins.queue = f"qPoolDynamic{qi}"
        return bi

    dp = nc.declare_dram_parameter
    xT_e = dp("xT", [P, SP1], FP32, isOutput=False)
    deg1_e = dp("deg1", [P, CH1], FP32, isOutput=False)
    deg2_e = dp("deg2", [P, CH2], FP32, isOutput=False)
    deg3_e = dp("deg3", [P, CH3], FP32, isOutput=False)
    NI1q = meta["NI1q"]
    idxq1_e = dp("idxq1", [P, NI1q * 8], I16, isOutput=False)
    mq1_e = dp("mq1", [P, 4 * NI1q], BF16, isOutput=False)
    idx2_e = dp("idx2", [P, NI2], I32, isOutput=False)
    idx3_e = dp("idx3", [P, NI3], I32, isOutput=False)
    r1_e = dp("r1", [P, 1], I32, isOutput=False)
    r2_e = dp("r2", [P, 1], I32, isOutput=False)
    W1_e = dp("W1", [P, F], FP32, isOutput=False)
    W2_e = dp("W2", [F, F], FP32, isOutput=False)
    W3_e = dp("W3", [F, F], FP32, isOutput=False)
    b1_e = dp("b1r", [P, F], FP32, isOutput=False)
    b2_e = dp("b2r", [P, F], FP32, isOutput=False)
    b3_e = dp("b3r", [P, F], FP32, isOutput=False)
    l1w_e = dp("l1w", [96, P], FP32, isOutput=False)
    l1b_e = dp("l1br", [P, P], FP32, isOutput=False)
    l2w_e = dp("l2w", [P, 2], FP32, isOutput=False)
    l2b_e = dp("l2br", [P, 2], FP32, isOutput=False)
    out_e = dp("out", [P, 2], FP32, isOutput=True)

    stripe1 = nc.dram_tensor("stripe1", [SP1, F], BF16)
    stripe2 = nc.dram_tensor("stripe2", [SP1, F], BF16)
    stripe3 = nc.dram_tensor("stripe3", [SP2, F], BF16)
    table1 = nc.dram_tensor("table1", [T1_ROWS, F], BF16, addr_space="Shared")
    table2 = nc.dram_tensor("table2", [T1_ROWS, F], BF16, addr_space="Shared")
    table3 = nc.dram_tensor("table3", [T3_ROWS, F], BF16, addr_space="Shared")
    h1_d = nc.dram_tensor("h1_d", [SP1, F], FP32)
    h2_d = nc.dram_tensor("h2_d", [SP2, F], FP32)

    with tile_mod.TileContext(nc) as tc:
        with (
            tc.tile_pool(name="pp", bufs=1) as pp,
            tc.tile_pool(name="gp", bufs=2) as gp,
            tc.tile_pool(name="tp", bufs=3) as tp,
        ):
            ident = pp.tile([P, P], FP32, tag="ident")
            make_identity(nc, ident[:])
            zrow = pp.tile([P, F], BF16, tag="zrow")
            nc.vector.memset(zrow[:], 0.0)
            for tbl, rows in ((table1, NC * SP1), (table2, NC * SP1), (table3, NC * SP2)):
                nc.sync.dma_start(out=tbl[rows : rows + P, :], in_=zrow[:])

            def load(ext, shape, tag, dt=FP32):
                t = pp.tile(shape, dt, tag=tag)
                nc.sync.dma_start(out=t[:], in_=ext[:])
                return t

            nc.gpsimd.load_library(library_config.mlp)
            # canonical padded chunk sizes -> 5 num_idxs regs total
            nidx_regs = {}
            for cc_ in (4, 8, 16, 32, 64):
                r_ = nc.alloc_register(mybir.EngineType.Pool, f"nidx{cc_}")
                nc.gpsimd.reg_mov(r_, cc_ * P)
                nidx_regs[cc_ * P] = r_
            xT = load(xT_e, [P, SP1], "xT")
            idxq1 = load(idxq1_e, [P, NI1q * 8], "idxq1", I16)
            mq1 = load(mq1_e, [P, 4 * NI1q], "mq1", BF16)
            idx2 = load(idx2_e, [P, NI2], "idx2", I32)
            idx3 = load(idx3_e, [P, NI3], "idx3", I32)
            r1i = load(r1_e, [P, 1], "r1i", I32)
            r2i = load(r2_e, [P, 1], "r2i", I32)
            W1 = load(W1_e, [P, F], "W1")
            W2 = load(W2_e, [F, F], "W2")
            W3 = load(W3_e, [F, F], "W3")
            b1 = load(b1_e, [P, F], "b1")
            b2 = load(b2_e, [P, F], "b2")
            b3 = load(b3_e, [P, F], "b3")
            l1w = load(l1w_e, [96, P], "l1w")
            l1b = load(l1b_e, [P, P], "l1b")
            l2w = load(l2w_e, [P, 2], "l2w")
            l2b = load(l2b_e, [P, 2], "l2b")

            def dis_of(ext, ch, tag):
                d = load(ext, [P, ch], tag)
                sq = tp.tile([P, ch], FP32, tag="sq")
                nc.scalar.sqrt(out=sq[:], in_=d[:])
                dis = pp.tile([P, ch], FP32, tag=tag + "_dis")
                nc.vector.reciprocal(out=dis[:], in_=sq[:])
                return dis

            dis1 = dis_of(deg1_e, CH1, "deg1")
            dis2 = dis_of(deg2_e, CH2, "deg2")
            dis3 = dis_of(deg3_e, CH3, "deg3")

            def bc_mid(ap2d, nch, width=F):
                # [128, nch] -> [128, nch, width] (inner bcast)
                return ap2d.rearrange("p (c o) -> p c o", o=1).to_broadcast(
                    [P, nch, width]
                )

            def bc_feat(ap2d, nch):
                # [128, F] -> [128, nch, F] (middle bcast)
                return ap2d.rearrange("p (o f) -> p o f", o=1).to_broadcast(
                    [P, nch, F]
                )

            with tc.tile_pool(name="ps", bufs=2, space="PSUM") as ps:

                def transform(kind, in_buf, n_chunks, W, dis, hs_tag):
                    """hs = dis * (h @ W); in_buf node-major (or xT for kind='x')."""
                    hs = pp.tile([P, n_chunks * F], BF16, tag=hs_tag)
                    for g0 in range(0, n_chunks, 4):
                        nch = min(4, n_chunks - g0)
                        if kind == "x":
                            zT_ps = ps.tile([F, 512], FP32, tag="zT", space="PSUM")
                            nc.tensor.matmul(
                                out=zT_ps[:, : nch * P],
                                lhsT=W[:],
                                rhs=in_buf[:, g0 * P : (g0 + nch) * P],
                                start=True, stop=True,
                            )
                        else:
                            hT_ps = ps.tile([F, 512], FP32, tag="hT", space="PSUM")
                            for k in range(nch):
                                nc.tensor.transpose(
                                    out=hT_ps[:, k * P : (k + 1) * P],
                                    in_=in_buf[:, (g0 + k) * F : (g0 + k + 1) * F],
                                    identity=ident[:],
                                )
                            hT_sb = tp.tile([F, 512], FP32, tag="hT_sb")
                            nc.scalar.copy(
                                out=hT_sb[:, : nch * P], in_=hT_ps[:, : nch * P]
                            )
                            zT_ps = ps.tile([F, 512], FP32, tag="zT", space="PSUM")
                            nc.tensor.matmul(
                                out=zT_ps[:, : nch * P],
                                lhsT=W[:],
                                rhs=hT_sb[:, : nch * P],
                                start=True, stop=True,
                            )
                        zT_sb = tp.tile([F, 512], FP32, tag="zT_sb")
                        nc.scalar.copy(out=zT_sb[:, : nch * P], in_=zT_ps[:, : nch * P])
                        zN_ps = ps.tile([P, 4 * F], FP32, tag="zN", space="PSUM")
                        for k in range(nch):
                            nc.tensor.transpose(
                                out=zN_ps[:, k * F : (k + 1) * F],
                                in_=zT_sb[:, k * P : (k + 1) * P],
                                identity=ident[:F, :F],
                            )
                        nc.vector.tensor_tensor(
                            out=hs[:, g0 * F : (g0 + nch) * F].rearrange(
                                "p (c f) -> p c f", c=nch
                            ),
                            in0=zN_ps[:, : nch * F].rearrange("p (c f) -> p c f", c=nch),
                            in1=bc_mid(dis[:, g0 : g0 + nch], nch),
                            op=ALU.mult,
                        )
                    return hs

                def broadcast(hs, stripe, table, n_chunks, rows):
                    nc.sync.dma_start(
                        out=stripe[:].rearrange("(p c) f -> p (c f)", p=P),
                        in_=hs[:, : n_chunks * F],
                    )
                    if skip_collectives:
                        return
                    nc.gpsimd.collective_compute(
                        "AllGather",
                        ALU.bypass,
                        replica_groups=[list(range(NC))],
                        ins=[stripe[:]],
                        outs=[table[0:rows, :]],
                    )

                def aggregate(A, table, idxt, cols_list, init_zero):
                    if init_zero:
                        nc.vector.memset(A[:], 0.0)
                    off = 0
                    for cols in cols_list:
                        g = gp.tile([P, CH1 * F], BF16, tag="g")
                        if skip_gathers:
                            nc.vector.memset(g[:, : cols * F], 0.0)
                        for ch in range(cols):
                            if skip_gathers:
                                continue
                            _q(nc.gpsimd.indirect_dma_start(
                                out=g[:, ch * F : (ch + 1) * F],
                                out_offset=None,
                                in_=table[:],
                                in_offset=bass.IndirectOffsetOnAxis(
                                    ap=idxt[:, off + ch : off + ch + 1], axis=0
                                ),
                            ))
                        nc.vector.tensor_add(
                            out=A[:, : cols * F],
                            in0=A[:, : cols * F],
                            in1=g[:, : cols * F],
                        )
                        off += cols

                def aggregate_q(A, tableq, idxq, mq, chunks, ni):
                    """Gather via dma_gather ucode: idx = row>>2 into 256B
                    quad-cells; 0/1 quadrant masks select the valid 32.
                    chunks: per round, tuples of (padded_cols, real_cols);
                    padded tail columns gather the zero row and are not added."""
                    off = 0  # column offset in the padded idx/mask arrays
                    for round_chunks in chunks:
                        done = 0  # real accumulator column offset this round
                        for ccp, cc in round_chunks:
                            nidx = ccp * P
                            gq = gp.tile([P, 64 * P], BF16, tag="gq")
                            nc.gpsimd.dma_gather(
                                gq[:, : ccp * P].rearrange("p (c e) -> p c e", e=P),
                                tableq,
                                idxq[:, off * 8 : (off + ccp) * 8],
                                nidx,
                                nidx_regs[nidx],
                                P,
                                single_packet=False,
                            )
                            g3 = gq[:, : cc * P].rearrange("p (c e) -> p c e", e=P)
                            a3 = A[:, done * F : (done + cc) * F].rearrange(
                                "p (c f) -> p c f", c=cc
                            )
                            for k in range(4):
                                tmp = tp.tile([P, 64 * F], BF16, tag="qtmp")
                                t3 = tmp[:, : cc * F].rearrange(
                                    "p (c f) -> p c f", c=cc
                                )
                                nc.vector.tensor_tensor(
                                    out=t3,
                                    in0=g3[:, :, k * F : (k + 1) * F],
                                    in1=bc_mid(
                                        mq[:, k * ni + off : k * ni + off + cc], cc
                                    ),
                                    op=ALU.mult,
                                )
                                nc.vector.tensor_tensor(
                                    out=a3, in0=a3, in1=t3, op=ALU.add
                                )
                            done += cc
                            off += ccp

                def finish(A, dis, b, n_chunks, h_tag):
                    A3d = A[:].rearrange("p (c f) -> p c f", c=n_chunks)
                    nc.vector.tensor_tensor(
                        out=A3d, in0=A3d, in1=bc_mid(dis[:], n_chunks), op=ALU.mult
                    )
                    nc.vector.tensor_tensor(
                        out=A3d, in0=A3d, in1=bc_feat(b[:], n_chunks), op=ALU.add
                    )
                    h = pp.tile([P, n_chunks * F], FP32, tag=h_tag)
                    nc.scalar.activation(out=h[:], in_=A[:], func=AF.Tanh)
                    return h

                # ---- layer 1 ----
                hs1 = transform("x", xT, CH1, W1, dis1, "hs1")
                broadcast(hs1, stripe1, table1, CH1, NC * SP1)
                A1 = pp.tile([P, CH1 * F], FP32, tag="A1")
                nc.vector.tensor_copy(out=A1[:], in_=hs1[:])
                if not skip_gathers:
                    aggregate_q(
                        A1,
                        table1[:].rearrange("(q a) f -> q (a f)", a=4),
                        idxq1, mq1, meta["chunks1"], NI1q,
                    )
                h1 = finish(A1, dis1, b1, CH1, "h1")
                nc.sync.dma_start(
                    out=h1_d[:].rearrange("(p c) f -> p (c f)", p=P), in_=h1[:]
                )

                # ---- layer 2 ----
                hs2 = transform("h", h1, CH1, W2, dis1, "hs2")
                broadcast(hs2, stripe2, table2, CH1, NC * SP1)
                A2 = pp.tile([P, CH2 * F], FP32, tag="A2")
                aggregate(A2, table2, idx2, meta["cols2"], init_zero=True)
                h2 = finish(A2, dis2, b2, CH2, "h2")
                nc.sync.dma_start(
                    out=h2_d[:].rearrange("(p c) f -> p (c f)", p=P), in_=h2[:]
                )

                # ---- layer 3 ----
                hs3 = transform("h", h2, CH2, W3, dis2, "hs3")
                broadcast(hs3, stripe3, table3, CH2, NC * SP2)
                A3 = pp.tile([P, CH3 * F], FP32, tag="A3")
                aggregate(A3, table3, idx3, meta["cols3"], init_zero=True)
                h3 = finish(A3, dis3, b3, CH3, "h3")

            # ---- readout ----
            with (
                tc.tile_pool(name="rp", bufs=1, space="PSUM") as rp,
                tc.tile_pool(name="rsb", bufs=1) as rsb,
            ):
                cat = rsb.tile([P, 96], FP32, tag="cat")
                nc.gpsimd.indirect_dma_start(
                    out=cat[:, 0:F], out_offset=None, in_=h1_d[:],
                    in_offset=bass.IndirectOffsetOnAxis(ap=r1i[:, 0:1], axis=0),
                )
                nc.gpsimd.indirect_dma_start(
                    out=cat[:, F : 2 * F], out_offset=None, in_=h2_d[:],
                    in_offset=bass.IndirectOffsetOnAxis(ap=r2i[:, 0:1], axis=0),
                )
                nc.vector.tensor_copy(out=cat[:, 2 * F : 3 * F], in_=h3[:, :F])

                cT_ps = rp.tile([96, P], FP32, tag="cT", space="PSUM")
                nc.tensor.transpose(out=cT_ps[:], in_=cat[:], identity=ident[:])
                cT = rsb.tile([96, P], FP32, tag="cTs")
                nc.scalar.copy(out=cT[:], in_=cT_ps[:])
                hid_ps = rp.tile([P, P], FP32, tag="hid", space="PSUM")
                nc.tensor.matmul(out=hid_ps[:], lhsT=cT[:], rhs=l1w[:], start=True, stop=True)
                hid = rsb.tile([P, P], FP32, tag="hids")
                nc.vector.tensor_add(out=hid[:], in0=hid_ps[:], in1=l1b[:])
                hidr = rsb.tile([P, P], FP32, tag="hidr")
                nc.scalar.activation(out=hidr[:], in_=hid[:], func=AF.Relu)
                hT_ps = rp.tile([P, P], FP32, tag="hT2", space="PSUM")
                nc.tensor.transpose(out=hT_ps[:], in_=hidr[:], identity=ident[:])
                hT = rsb.tile([P, P], FP32, tag="hT2s")
                nc.scalar.copy(out=hT[:], in_=hT_ps[:])
                lg_ps = rp.tile([P, 2], FP32, tag="lg", space="PSUM")
                nc.tensor.matmul(out=lg_ps[:], lhsT=hT[:], rhs=l2w[:], start=True, stop=True)
                lg = rsb.tile([P, 2], FP32, tag="lgs")
                nc.vector.tensor_add(out=lg[:], in0=lg_ps[:], in1=l2b[:])
                m = rsb.tile([P, 1], FP32, tag="m")
                nc.vector.tensor_reduce(out=m[:], in_=lg[:], axis=mybir.AxisListType.X, op=ALU.max)
                t = rsb.tile([P, 2], FP32, tag="t")
                nc.vector.tensor_scalar(out=t[:], in0=lg[:], scalar1=m[:], scalar2=None, op0=ALU.subtract)
                e = rsb.tile([P, 2], FP32, tag="e")
                nc.scalar.activation(out=e[:], in_=t[:], func=AF.Exp)
                s = rsb.tile([P, 1], FP32, tag="s")
                nc.vector.tensor_reduce(out=s[:], in_=e[:], axis=mybir.AxisListType.X, op=ALU.add)
                ls = rsb.tile([P, 1], FP32, tag="ls")
                nc.scalar.activation(out=ls[:], in_=s[:], func=AF.Ln)
                o = rsb.tile([P, 2], FP32, tag="o")
                nc.vector.tensor_scalar(out=o[:], in0=t[:], scalar1=ls[:], scalar2=None, op0=ALU.subtract)
                nc.sync.dma_start(out=out_e[:], in_=o[:])

    _split_waits(nc)
    mybir.codegen_inst_isa_subclasses(nc)
    return nc


# ---------------------------------------------------------------------------
# entry point
# ---------------------------------------------------------------------------

_CACHE = {}


def _get_runner(meta):
    key = (
        meta["CH1"], meta["CH2"], meta["CH3"], meta["NI1"], meta["NI2"], meta["NI3"],
        tuple(meta["cols1"]), tuple(meta["cols2"]), tuple(meta["cols3"]),
    )
    if key not in _CACHE:
        nc = _build(meta)
        _CACHE[key] = _SpmdRunner(nc)
    return _CACHE[key]


def kernel(x, edge_index, batch, num_graphs,
           W1, b1, W2, b2, W3, b3, lin1_w, lin1_b, lin2_w, lin2_b):
    x = np.asarray(x, np.float32)
    edge_index = np.asarray(edge_index)
    batch = np.asarray(batch)
    G = int(np.asarray(num_graphs))
    W1 = np.asarray(W1, np.float32)
    W2 = np.asarray(W2, np.float32)
    W3 = np.asarray(W3, np.float32)
    b1 = np.asarray(b1, np.float32)
    b2 = np.asarray(b2, np.float32)
    b3 = np.asarray(b3, np.float32)
    lin1_w = np.asarray(lin1_w, np.float32)
    lin1_b = np.asarray(lin1_b, np.float32)
    lin2_w = np.asarray(lin2_w, np.float32)
    lin2_b = np.asarray(lin2_b, np.float32)

    meta, per_core, slot_graphs = _preprocess(x, edge_index, batch, G)
    runner = _get_runner(meta)

    in_maps = []
    for c in range(NC):
        pc = per_core[c]
        in_maps.append(
            dict(
                xT=pc["xT"],
                deg1=pc["deg1"], deg2=pc["deg2"], deg3=pc["deg3"],
                idxq1=pc["idxq1"], mq1=pc["mq1"],
                idx2=pc["idx2"], idx3=pc["idx3"],
                r1=pc["r1"], r2=pc["r2"],
                W1=W1, W2=W2, W3=W3,
                b1r=np.tile(b1[None, :], (P, 1)),
                b2r=np.tile(b2[None, :], (P, 1)),
                b3r=np.tile(b3[None, :], (P, 1)),
                l1w=lin1_w.astype(np.float32),
                l1br=np.tile(lin1_b[None, :], (P, 1)),
                l2w=lin2_w.astype(np.float32),
                l2br=np.tile(lin2_b[None, :], (P, 1)),
            )
        )

    args = runner.stage(in_maps)
    outs = runner.run_staged(args)
    res = runner.results(outs)

    logits = np.zeros((G, 2), np.float32)
    for c in range(NC):
        gids = slot_graphs[c]
        logits[gids] = res[c]["out"][: len(gids)]

    # expose for test.py timing
    kernel._last = (runner, args)
    return logits

